# revision 14
# baseline (speedup 1.0000x reference)
"""Trainium2 Bass kernel for the EnhancedFashionRecommender module.

Strategy (8 NeuronCores, row-sharded user table):
  - The batch touches 8192 distinct users ("hot" rows); the remaining
    91808 users are "cold".  Each core owns 1024 hot users (its slice of
    the batch, so routing is free) plus 11476 cold users.
  - Hot rows: gather + curriculum blend + masked-mean update + momentum
    blend + l2norm, all on-device, batch-items-on-partitions layout.
  - Cold rows: straight DRAM->DRAM pass-through on the second HWDGE ring
    so it streams concurrently with the hot pipeline.
  - shared_prototypes update: per-core partial sum of the normalized
    updates, AllReduce'd across the 8 cores, then blended on-device.
  - Host does only data routing: slicing, layout transposes, and
    scatter of the per-core outputs back into full-shape arrays.
"""
import numpy as np

N_CORES = 8
B, S, D, P, U = 8192, 50, 128, 16, 100000
BC = B // N_CORES            # 1024 batch items per core
T = BC // 128                # 8 tiles of 128 items
PD = P * D                   # 2048
NCOLD = (U - B) // N_CORES   # 11476 cold rows per core
MOM = 0.9

_NC = None          # cached compiled Bass module
LAST_RESULTS = None  # BassKernelResults of the most recent run (for test.py)


def _build(include_cold=True, include_cc=True):
    import concourse.bacc as bacc
    import concourse.tile as tile
    from concourse import mybir

    f32 = mybir.dt.float32
    AX = mybir.AxisListType
    OP = mybir.AluOpType
    AF = mybir.ActivationFunctionType

    nc = bacc.Bacc("TRN2", target_bir_lowering=False, debug=False,
                   num_devices=N_CORES)

    feat_h = nc.declare_dram_parameter("feat", [T, 128, D, S], f32, isOutput=False)
    sm_h = nc.declare_dram_parameter("sm", [128, T, S], f32, isOutput=False)
    cnt_h = nc.declare_dram_parameter("cnt", [128, T], f32, isOutput=False)
    up_h = nc.declare_dram_parameter("up", [T, 128, PD], f32, isOutput=False)
    cold_h = nc.declare_dram_parameter("cold", [NCOLD, PD], f32, isOutput=False)
    ccnt_h = nc.declare_dram_parameter("ccnt", [NCOLD], f32, isOutput=False)
    sh_h = nc.declare_dram_parameter("sh", [P, D], f32, isOutput=False)
    shrep_h = nc.declare_dram_parameter("shrep", [128, PD], f32, isOutput=False)

    comb_o = nc.declare_dram_parameter("comb_o", [T, 128, PD], f32, isOutput=True)
    newr_o = nc.declare_dram_parameter("newr_o", [T, 128, PD], f32, isOutput=True)
    cold_o = nc.declare_dram_parameter("cold_o", [NCOLD, PD], f32, isOutput=True)
    ccnt_o = nc.declare_dram_parameter("ccnt_o", [NCOLD], f32, isOutput=True)
    cnt_o = nc.declare_dram_parameter("cnt_o", [128, T], f32, isOutput=True)
    sh_o = nc.declare_dram_parameter("sh_o", [P, D], f32, isOutput=True)

    with tile.TileContext(nc) as tc:
        with tc.tile_pool(name="setup", bufs=1) as setup, \
             tc.tile_pool(name="big", bufs=2) as big, \
             tc.tile_pool(name="mid", bufs=2) as mid, \
             tc.tile_pool(name="small", bufs=2) as small, \
             tc.tile_pool(name="dram", bufs=1, space="DRAM") as dram:

            # Cold pass-through on the ACT HWDGE ring so it streams
            # concurrently with the SP-ring hot pipeline below.
            if include_cold:
                nc.scalar.dma_start(out=cold_o[:], in_=cold_h[:])
                nc.scalar.dma_start(out=ccnt_o[:], in_=ccnt_h[:])

            # Whole-core setup loads.
            sm_sb = setup.tile([128, T, S], f32)
            nc.sync.dma_start(out=sm_sb[:], in_=sm_h[:])
            cnt_sb = setup.tile([128, T], f32)
            nc.sync.dma_start(out=cnt_sb[:], in_=cnt_h[:])
            shflat_sb = setup.tile([1, PD], f32)   # shared flat on partition 0
            nc.sync.dma_start(out=shflat_sb[:],
                              in_=sh_h[:].flatten().unsqueeze(0))
            shbc = setup.tile([128, PD], f32)      # host-replicated shared
            nc.sync.dma_start(out=shbc[:], in_=shrep_h[:])
            acc = setup.tile([128, 128], f32)
            nc.vector.memset(acc[:], 0.0)

            # Per-item scalars for the whole core at once ([128, T]).
            w_arg = setup.tile([128, T], f32)    # cnt/100 - 3
            nc.vector.tensor_scalar(w_arg[:], cnt_sb[:], 0.01, -3.0,
                                    OP.mult, OP.add)
            w_sb = setup.tile([128, T], f32)     # sigmoid(cnt/100 - 3)
            nc.scalar.activation(w_sb[:], w_arg[:], AF.Sigmoid)
            omw_sb = setup.tile([128, T], f32)   # 1 - w
            nc.vector.tensor_scalar(omw_sb[:], w_sb[:], -1.0, 1.0,
                                    OP.mult, OP.add)
            m_sb = setup.tile([128, T], f32)     # min(0.9 + cnt/1000, 0.99)
            nc.vector.tensor_scalar(m_sb[:], cnt_sb[:], 0.001, MOM,
                                    OP.mult, OP.add)
            nc.vector.tensor_scalar_min(m_sb[:], m_sb[:], 0.99)
            omm_sb = setup.tile([128, T], f32)   # 1 - m
            nc.vector.tensor_scalar(omm_sb[:], m_sb[:], -1.0, 1.0,
                                    OP.mult, OP.add)
            cnt1 = setup.tile([128, T], f32)     # cnt + 1
            nc.vector.tensor_scalar(cnt1[:], cnt_sb[:], 1.0, None, OP.add)
            nc.sync.dma_start(out=cnt_o[:], in_=cnt1[:])

            for t in range(T):
                ft = big.tile([128, D, S], f32, tag="ft")
                nc.sync.dma_start(out=ft[:], in_=feat_h[t])
                upt = mid.tile([128, PD], f32, tag="upt")
                nc.sync.dma_start(out=upt[:], in_=up_h[t])

                # upd = sum_s(features * mask) / max(sum_s(mask), 1e-6)
                mview = sm_sb[:, t, :].unsqueeze(1).broadcast_to([128, D, S])
                nc.vector.tensor_mul(ft[:], ft[:], mview)
                u = small.tile([128, 128], f32, tag="u")
                nc.vector.tensor_reduce(u[:], ft[:], axis=AX.X, op=OP.add)
                msum = small.tile([128, 1], f32, tag="msum")
                nc.vector.tensor_reduce(msum[:], sm_sb[:, t, :], axis=AX.X,
                                        op=OP.add)
                msc = small.tile([128, 1], f32, tag="msc")
                nc.vector.tensor_scalar_max(msc[:], msum[:], 1e-6)
                rden = small.tile([128, 1], f32, tag="rden")
                nc.vector.reciprocal(rden[:], msc[:])
                upd = small.tile([128, 128], f32, tag="upd")
                nc.vector.tensor_scalar(upd[:], u[:], rden[:], None, OP.mult)

                # l2norm: upd / max(||upd||, 1e-12)
                sqscr = small.tile([128, 128], f32, tag="sqscr")
                ssq = small.tile([128, 1], f32, tag="ssq")
                nc.scalar.activation(sqscr[:], upd[:], AF.Square,
                                     accum_out=ssq[:])
                nrm = small.tile([128, 1], f32, tag="nrm")
                nc.scalar.sqrt(nrm[:], ssq[:])
                nrc = small.tile([128, 1], f32, tag="nrc")
                nc.vector.tensor_scalar_max(nrc[:], nrm[:], 1e-12)
                rno = small.tile([128, 1], f32, tag="rno")
                nc.vector.reciprocal(rno[:], nrc[:])
                updn = small.tile([128, 128], f32, tag="updn")
                nc.vector.tensor_scalar(updn[:], upd[:], rno[:], None, OP.mult)
                nc.vector.tensor_add(acc[:], acc[:], updn[:])

                # combined = w*user_p + (1-w)*shared
                tmp = mid.tile([128, PD], f32, tag="tmp")
                nc.vector.tensor_scalar(tmp[:], shbc[:], omw_sb[:, t:t + 1],
                                        None, OP.mult)
                comb = mid.tile([128, PD], f32, tag="comb")
                nc.vector.scalar_tensor_tensor(
                    out=comb[:], in0=upt[:], scalar=w_sb[:, t:t + 1],
                    in1=tmp[:], op0=OP.mult, op1=OP.add)
                nc.sync.dma_start(out=comb_o[t], in_=comb[:])

                # new_rows = m*user_p + (1-m)*upd_broadcast
                updn_b = updn[:].unsqueeze(1).broadcast_to([128, P, D])
                tmp2 = mid.tile([128, P, D], f32, tag="tmp2")
                nc.vector.tensor_scalar(tmp2[:], updn_b, omm_sb[:, t:t + 1],
                                        None, OP.mult)
                newr = mid.tile([128, P, D], f32, tag="newr")
                upt3 = upt[:].rearrange("p (a b) -> p a b", a=P)
                nc.vector.scalar_tensor_tensor(
                    out=newr[:], in0=upt3, scalar=m_sb[:, t:t + 1],
                    in1=tmp2[:], op0=OP.mult, op1=OP.add)
                nc.sync.dma_start(out=newr_o[t],
                                  in_=newr[:].rearrange("p a b -> p (a b)"))

            # shared update: AllReduce the column sums of updn over cores.
            # Cross-partition sum: rebase+fold 128->32, then 4 32x32 block
            # transposes (out bases 0/32/64/96), then a free-dim reduce.
            c1 = setup.tile([64, 128], f32)
            nc.vector.tensor_copy(c1[:], acc[64:128, :])
            f64 = setup.tile([64, 128], f32)
            nc.vector.tensor_add(f64[:], acc[0:64, :], c1[:])
            c2 = setup.tile([32, 128], f32)
            nc.vector.tensor_copy(c2[:], f64[32:64, :])
            f32t = setup.tile([32, 128], f32)
            nc.vector.tensor_add(f32t[:], f64[0:32, :], c2[:])
            accT = setup.tile([128, 32], f32)
            for j in range(4):
                nc.vector.transpose(accT[32 * j:32 * j + 32, :],
                                    f32t[:, 32 * j:32 * j + 32])
            s_d = setup.tile([128, 1], f32)
            nc.vector.tensor_reduce(s_d[:], accT[:], axis=AX.X, op=OP.add)
            cc_in = dram.tile([1, 128], f32, name="cc_in")
            cc_out = dram.tile([1, 128], f32, addr_space="Shared", name="cc_out")
            # SWDGE (gpsimd) ring for the collective path: keeps its
            # completion semaphores off the HWDGE lanes used by the big
            # streaming DMAs, so the AllReduce can overlap the cold copy.
            nc.gpsimd.dma_start(out=cc_in[:], in_=s_d[:])
            if include_cc:
                nc.gpsimd.collective_compute(
                    "AllReduce", OP.add,
                    replica_groups=[list(range(N_CORES))],
                    ins=[cc_in.opt()], outs=[cc_out.opt()])
            else:
                nc.gpsimd.dma_start(out=cc_out[:], in_=cc_in[:])
            gsum = setup.tile([1, 128], f32)
            nc.gpsimd.dma_start(out=gsum[:], in_=cc_out[:])
            mean = setup.tile([1, 128], f32)
            nc.vector.tensor_scalar(mean[:], gsum[:], 1.0 / B, None, OP.mult)
            sq2 = setup.tile([1, 128], f32)
            ss2 = setup.tile([1, 1], f32)
            nc.scalar.activation(sq2[:], mean[:], AF.Square,
                                 accum_out=ss2[:])
            nr2 = setup.tile([1, 1], f32)
            nc.scalar.sqrt(nr2[:], ss2[:])
            nc2t = setup.tile([1, 1], f32)
            nc.vector.tensor_scalar_max(nc2t[:], nr2[:], 1e-12)
            rn2 = setup.tile([1, 1], f32)
            nc.vector.reciprocal(rn2[:], nc2t[:])
            srow = setup.tile([1, 128], f32)
            nc.vector.tensor_scalar(srow[:], mean[:], rn2[:], None, OP.mult)
            # new_shared = 0.9*shared + 0.1*srow, on partition 0, flat layout
            srow_rep = srow[:].unsqueeze(1).broadcast_to([1, P, D])
            tmp4 = setup.tile([1, P, D], f32)
            nc.vector.tensor_scalar(tmp4[:], srow_rep, 1.0 - MOM, None,
                                    OP.mult)
            shnew = setup.tile([1, PD], f32)
            nc.vector.scalar_tensor_tensor(
                out=shnew[:].rearrange("p (a b) -> p a b", a=P),
                in0=shflat_sb[:].rearrange("p (a b) -> p a b", a=P),
                scalar=MOM, in1=tmp4[:], op0=OP.mult, op1=OP.add)
            nc.gpsimd.dma_start(out=sh_o[:].flatten().unsqueeze(0),
                                in_=shnew[:])

    nc.compile()
    return nc


def kernel(user_idx, features, success_mask, user_prototypes,
           shared_prototypes, interaction_count):
    global _NC, LAST_RESULTS
    import concourse.bass_utils as bass_utils

    user_idx = np.asarray(user_idx)
    features = np.asarray(features, dtype=np.float32)
    success_mask = np.asarray(success_mask, dtype=np.float32)
    user_prototypes = np.asarray(user_prototypes, dtype=np.float32)
    shared_prototypes = np.asarray(shared_prototypes, dtype=np.float32)
    interaction_count = np.asarray(interaction_count, dtype=np.float32)

    hot = user_idx.astype(np.int64)
    assert hot.shape == (B,) and np.unique(hot).size == B, \
        "kernel assumes unique user indices"
    cold_mask = np.ones(U, dtype=bool)
    cold_mask[hot] = False
    cold = np.nonzero(cold_mask)[0]
    cold_splits = np.array_split(cold, N_CORES)

    shared2d = np.ascontiguousarray(shared_prototypes.reshape(P, D))

    in_maps = []
    for k in range(N_CORES):
        sl = slice(k * BC, (k + 1) * BC)
        hot_k = hot[sl]
        ck = cold_splits[k]
        feat_t = np.ascontiguousarray(
            features[sl].transpose(0, 2, 1)).reshape(T, 128, D, S)
        sm_k = np.ascontiguousarray(
            success_mask[sl].reshape(T, 128, S).transpose(1, 0, 2))
        cnt_k = np.ascontiguousarray(
            interaction_count[hot_k].reshape(T, 128).T)
        up_k = user_prototypes[hot_k].reshape(T, 128, PD)
        cold_k = user_prototypes[ck].reshape(NCOLD, PD)
        ccnt_k = interaction_count[ck]
        in_maps.append({
            "feat": feat_t, "sm": sm_k, "cnt": cnt_k,
            "up": np.ascontiguousarray(up_k),
            "cold": np.ascontiguousarray(cold_k),
            "ccnt": np.ascontiguousarray(ccnt_k),
            "sh": shared2d,
            "shrep": np.ascontiguousarray(
                np.broadcast_to(shared2d.reshape(1, PD), (128, PD))),
        })

    if _NC is None:
        _NC = _build()
    res = bass_utils.run_bass_kernel_spmd(_NC, in_maps, list(range(N_CORES)))
    LAST_RESULTS = res

    combined = np.empty((B, P, D), dtype=np.float32)
    new_rows = np.empty((B, P, D), dtype=np.float32)
    new_user_prototypes = np.empty((U, P, D), dtype=np.float32)
    new_interaction_count = np.empty(U, dtype=np.float32)
    for k in range(N_CORES):
        sl = slice(k * BC, (k + 1) * BC)
        out = res.results[k]
        combined[sl] = out["comb_o"].reshape(BC, P, D)
        new_rows[sl] = out["newr_o"].reshape(BC, P, D)
        new_user_prototypes[cold_splits[k]] = out["cold_o"].reshape(NCOLD, P, D)
        new_interaction_count[cold_splits[k]] = out["ccnt_o"]
        new_interaction_count[hot[sl]] = out["cnt_o"].T.reshape(BC)
    new_user_prototypes[hot] = new_rows
    new_shared = res.results[0]["sh_o"].reshape(1, P, D).copy()

    return (combined, new_rows, new_user_prototypes, new_shared,
            new_interaction_count)


# revision 21
# speedup vs baseline: 1.1150x; 1.1150x over previous
"""Trainium2 Bass kernel for the EnhancedFashionRecommender module.

Strategy (8 NeuronCores, row-sharded user table):
  - The batch touches 8192 distinct users ("hot" rows); the remaining
    91808 users are "cold".  Each core owns 1024 hot users (its slice of
    the batch, so routing is free) plus 11476 cold users.
  - Hot rows: gather + curriculum blend + masked-mean update + momentum
    blend + l2norm, all on-device, batch-items-on-partitions layout.
  - Cold rows: straight DRAM->DRAM pass-through on the second HWDGE ring
    so it streams concurrently with the hot pipeline.
  - shared_prototypes update: per-core partial sum of the normalized
    updates, AllReduce'd across the 8 cores, then blended on-device.
  - Host does only data routing: slicing, layout transposes, and
    scatter of the per-core outputs back into full-shape arrays.
"""
import numpy as np

N_CORES = 8
B, S, D, P, U = 8192, 50, 128, 16, 100000
BC = B // N_CORES            # 1024 batch items per core
T = BC // 128                # 8 tiles of 128 items
PD = P * D                   # 2048
NCOLD = (U - B) // N_CORES   # 11476 cold rows per core
MOM = 0.9

_NC = None          # cached compiled Bass module
LAST_RESULTS = None  # BassKernelResults of the most recent run (for test.py)


def _build(include_cold=True, include_cc=True):
    import concourse.bacc as bacc
    import concourse.tile as tile
    from concourse import mybir

    f32 = mybir.dt.float32
    AX = mybir.AxisListType
    OP = mybir.AluOpType
    AF = mybir.ActivationFunctionType

    nc = bacc.Bacc("TRN2", target_bir_lowering=False, debug=False,
                   num_devices=N_CORES)

    feat_h = nc.declare_dram_parameter("feat", [T, 128, D, S], f32, isOutput=False)
    sm_h = nc.declare_dram_parameter("sm", [128, T, S], f32, isOutput=False)
    cnt_h = nc.declare_dram_parameter("cnt", [128, T], f32, isOutput=False)
    up_h = nc.declare_dram_parameter("up", [T, 128, PD], f32, isOutput=False)
    cold_h = nc.declare_dram_parameter("cold", [NCOLD, PD], f32, isOutput=False)
    ccnt_h = nc.declare_dram_parameter("ccnt", [NCOLD], f32, isOutput=False)
    sh_h = nc.declare_dram_parameter("sh", [P, D], f32, isOutput=False)
    shrep_h = nc.declare_dram_parameter("shrep", [128, PD], f32, isOutput=False)

    comb_o = nc.declare_dram_parameter("comb_o", [T, 128, PD], f32, isOutput=True)
    newr_o = nc.declare_dram_parameter("newr_o", [T, 128, PD], f32, isOutput=True)
    cold_o = nc.declare_dram_parameter("cold_o", [NCOLD, PD], f32, isOutput=True)
    ccnt_o = nc.declare_dram_parameter("ccnt_o", [NCOLD], f32, isOutput=True)
    cnt_o = nc.declare_dram_parameter("cnt_o", [128, T], f32, isOutput=True)
    sh_o = nc.declare_dram_parameter("sh_o", [P, D], f32, isOutput=True)

    with tile.TileContext(nc) as tc:
        with tc.tile_pool(name="setup", bufs=1) as setup, \
             tc.tile_pool(name="big", bufs=3) as big, \
             tc.tile_pool(name="mid", bufs=2) as mid, \
             tc.tile_pool(name="small", bufs=2) as small, \
             tc.tile_pool(name="dram", bufs=1, space="DRAM") as dram:

            # Whole-core setup loads: tiny ones on the SWDGE (gpsimd) ring
            # so the SP HWDGE ring carries only the big streaming DMAs.
            sm_sb = setup.tile([128, T, S], f32)
            nc.gpsimd.dma_start(out=sm_sb[:], in_=sm_h[:])
            cnt_sb = setup.tile([128, T], f32)
            nc.gpsimd.dma_start(out=cnt_sb[:], in_=cnt_h[:])
            shflat_sb = setup.tile([1, PD], f32)   # shared flat on partition 0
            nc.gpsimd.dma_start(out=shflat_sb[:],
                                in_=sh_h[:].flatten().unsqueeze(0))
            shbc = setup.tile([128, PD], f32)      # host-replicated shared
            nc.sync.dma_start(out=shbc[:], in_=shrep_h[:])
            acc = setup.tile([128, 128], f32)
            nc.vector.memset(acc[:], 0.0)

            # Per-item scalars for the whole core at once ([128, T]).
            w_arg = setup.tile([128, T], f32)    # cnt/100 - 3
            nc.vector.tensor_scalar(w_arg[:], cnt_sb[:], 0.01, -3.0,
                                    OP.mult, OP.add)
            w_sb = setup.tile([128, T], f32)     # sigmoid(cnt/100 - 3)
            nc.scalar.activation(w_sb[:], w_arg[:], AF.Sigmoid)
            omw_sb = setup.tile([128, T], f32)   # 1 - w
            nc.vector.tensor_scalar(omw_sb[:], w_sb[:], -1.0, 1.0,
                                    OP.mult, OP.add)
            m_sb = setup.tile([128, T], f32)     # min(0.9 + cnt/1000, 0.99)
            nc.vector.tensor_scalar(m_sb[:], cnt_sb[:], 0.001, MOM,
                                    OP.mult, OP.add)
            nc.vector.tensor_scalar_min(m_sb[:], m_sb[:], 0.99)
            omm_sb = setup.tile([128, T], f32)   # 1 - m
            nc.vector.tensor_scalar(omm_sb[:], m_sb[:], -1.0, 1.0,
                                    OP.mult, OP.add)
            cnt1 = setup.tile([128, T], f32)     # cnt + 1
            nc.vector.tensor_scalar(cnt1[:], cnt_sb[:], 1.0, None, OP.add)
            nc.gpsimd.dma_start(out=cnt_o[:], in_=cnt1[:])

            # Cold pass-through rides the same SP ring, chunked and
            # interleaved between tiles: during each sequencer stall on a
            # compute wait, the ring keeps draining cold bytes, and the
            # tail chunks hide the collective + shared-update epilogue.
            CHUNK = 512
            cold_pos = 0

            def cold_chunk(rows):
                nonlocal cold_pos
                rows = min(rows, NCOLD - cold_pos)
                if include_cold and rows > 0:
                    nc.sync.dma_start(
                        out=cold_o[cold_pos:cold_pos + rows],
                        in_=cold_h[cold_pos:cold_pos + rows])
                    cold_pos += rows

            fts, upts = {}, {}

            def load_tile(t):
                if t >= T:
                    return
                fts[t] = big.tile([128, D, S], f32, tag="ft", name=f"ft{t}")
                nc.sync.dma_start(out=fts[t][:], in_=feat_h[t])
                upts[t] = mid.tile([128, PD], f32, tag="upt", bufs=3,
                                   name=f"upt{t}")
                nc.sync.dma_start(out=upts[t][:], in_=up_h[t])

            load_tile(0)
            load_tile(1)
            for t in range(T):
                ft, upt = fts.pop(t), upts.pop(t)

                # upd = sum_s(features * mask) / max(sum_s(mask), 1e-6)
                mview = sm_sb[:, t, :].unsqueeze(1).broadcast_to([128, D, S])
                nc.vector.tensor_mul(ft[:], ft[:], mview)
                u = small.tile([128, 128], f32, tag="u")
                nc.vector.tensor_reduce(u[:], ft[:], axis=AX.X, op=OP.add)
                msum = small.tile([128, 1], f32, tag="msum")
                nc.vector.tensor_reduce(msum[:], sm_sb[:, t, :], axis=AX.X,
                                        op=OP.add)
                msc = small.tile([128, 1], f32, tag="msc")
                nc.vector.tensor_scalar_max(msc[:], msum[:], 1e-6)
                rden = small.tile([128, 1], f32, tag="rden")
                nc.vector.reciprocal(rden[:], msc[:])
                upd = small.tile([128, 128], f32, tag="upd")
                nc.vector.tensor_scalar(upd[:], u[:], rden[:], None, OP.mult)

                # l2norm: upd / max(||upd||, 1e-12)
                sqscr = small.tile([128, 128], f32, tag="sqscr")
                ssq = small.tile([128, 1], f32, tag="ssq")
                nc.scalar.activation(sqscr[:], upd[:], AF.Square,
                                     accum_out=ssq[:])
                nrm = small.tile([128, 1], f32, tag="nrm")
                nc.scalar.sqrt(nrm[:], ssq[:])
                nrc = small.tile([128, 1], f32, tag="nrc")
                nc.vector.tensor_scalar_max(nrc[:], nrm[:], 1e-12)
                rno = small.tile([128, 1], f32, tag="rno")
                nc.vector.reciprocal(rno[:], nrc[:])
                updn = small.tile([128, 128], f32, tag="updn")
                nc.vector.tensor_scalar(updn[:], upd[:], rno[:], None, OP.mult)
                nc.vector.tensor_add(acc[:], acc[:], updn[:])

                # combined = w*user_p + (1-w)*shared
                tmp = mid.tile([128, PD], f32, tag="tmp", bufs=1)
                nc.vector.tensor_scalar(tmp[:], shbc[:], omw_sb[:, t:t + 1],
                                        None, OP.mult)
                comb = mid.tile([128, PD], f32, tag="comb")
                nc.vector.scalar_tensor_tensor(
                    out=comb[:], in0=upt[:], scalar=w_sb[:, t:t + 1],
                    in1=tmp[:], op0=OP.mult, op1=OP.add)
                nc.sync.dma_start(out=comb_o[t], in_=comb[:])

                # new_rows = m*user_p + (1-m)*upd_broadcast
                updn_b = updn[:].unsqueeze(1).broadcast_to([128, P, D])
                tmp2 = mid.tile([128, P, D], f32, tag="tmp2", bufs=1)
                nc.vector.tensor_scalar(tmp2[:], updn_b, omm_sb[:, t:t + 1],
                                        None, OP.mult)
                newr = mid.tile([128, P, D], f32, tag="newr")
                upt3 = upt[:].rearrange("p (a b) -> p a b", a=P)
                nc.vector.scalar_tensor_tensor(
                    out=newr[:], in0=upt3, scalar=m_sb[:, t:t + 1],
                    in1=tmp2[:], op0=OP.mult, op1=OP.add)
                nc.sync.dma_start(out=newr_o[t],
                                  in_=newr[:].rearrange("p a b -> p (a b)"))
                cold_chunk(CHUNK)
                load_tile(t + 2)

            # Remaining cold rows drain at the end, hiding the epilogue.
            if include_cold:
                while cold_pos < NCOLD:
                    cold_chunk(2048)
                nc.sync.dma_start(out=ccnt_o[:], in_=ccnt_h[:])

            # shared update: AllReduce the column sums of updn over cores.
            # Cross-partition sum: rebase+fold 128->32, then 4 32x32 block
            # transposes (out bases 0/32/64/96), then a free-dim reduce.
            c1 = setup.tile([64, 128], f32)
            nc.vector.tensor_copy(c1[:], acc[64:128, :])
            f64 = setup.tile([64, 128], f32)
            nc.vector.tensor_add(f64[:], acc[0:64, :], c1[:])
            c2 = setup.tile([32, 128], f32)
            nc.vector.tensor_copy(c2[:], f64[32:64, :])
            f32t = setup.tile([32, 128], f32)
            nc.vector.tensor_add(f32t[:], f64[0:32, :], c2[:])
            accT = setup.tile([128, 32], f32)
            for j in range(4):
                nc.vector.transpose(accT[32 * j:32 * j + 32, :],
                                    f32t[:, 32 * j:32 * j + 32])
            s_d = setup.tile([128, 1], f32)
            nc.vector.tensor_reduce(s_d[:], accT[:], axis=AX.X, op=OP.add)
            cc_in = dram.tile([1, 128], f32, name="cc_in")
            cc_out = dram.tile([1, 128], f32, addr_space="Shared", name="cc_out")
            # SWDGE (gpsimd) ring for the collective path: keeps its
            # completion semaphores off the HWDGE lanes used by the big
            # streaming DMAs, so the AllReduce can overlap the cold copy.
            nc.gpsimd.dma_start(out=cc_in[:], in_=s_d[:])
            if include_cc:
                nc.gpsimd.collective_compute(
                    "AllReduce", OP.add,
                    replica_groups=[list(range(N_CORES))],
                    ins=[cc_in.opt()], outs=[cc_out.opt()])
            else:
                nc.gpsimd.dma_start(out=cc_out[:], in_=cc_in[:])
            gsum = setup.tile([1, 128], f32)
            nc.gpsimd.dma_start(out=gsum[:], in_=cc_out[:])
            mean = setup.tile([1, 128], f32)
            nc.vector.tensor_scalar(mean[:], gsum[:], 1.0 / B, None, OP.mult)
            sq2 = setup.tile([1, 128], f32)
            ss2 = setup.tile([1, 1], f32)
            nc.scalar.activation(sq2[:], mean[:], AF.Square,
                                 accum_out=ss2[:])
            nr2 = setup.tile([1, 1], f32)
            nc.scalar.sqrt(nr2[:], ss2[:])
            nc2t = setup.tile([1, 1], f32)
            nc.vector.tensor_scalar_max(nc2t[:], nr2[:], 1e-12)
            rn2 = setup.tile([1, 1], f32)
            nc.vector.reciprocal(rn2[:], nc2t[:])
            srow = setup.tile([1, 128], f32)
            nc.vector.tensor_scalar(srow[:], mean[:], rn2[:], None, OP.mult)
            # new_shared = 0.9*shared + 0.1*srow, on partition 0, flat layout
            srow_rep = srow[:].unsqueeze(1).broadcast_to([1, P, D])
            tmp4 = setup.tile([1, P, D], f32)
            nc.vector.tensor_scalar(tmp4[:], srow_rep, 1.0 - MOM, None,
                                    OP.mult)
            shnew = setup.tile([1, PD], f32)
            nc.vector.scalar_tensor_tensor(
                out=shnew[:].rearrange("p (a b) -> p a b", a=P),
                in0=shflat_sb[:].rearrange("p (a b) -> p a b", a=P),
                scalar=MOM, in1=tmp4[:], op0=OP.mult, op1=OP.add)
            nc.gpsimd.dma_start(out=sh_o[:].flatten().unsqueeze(0),
                                in_=shnew[:])

    nc.compile()
    return nc


def kernel(user_idx, features, success_mask, user_prototypes,
           shared_prototypes, interaction_count):
    global _NC, LAST_RESULTS
    import concourse.bass_utils as bass_utils

    user_idx = np.asarray(user_idx)
    features = np.asarray(features, dtype=np.float32)
    success_mask = np.asarray(success_mask, dtype=np.float32)
    user_prototypes = np.asarray(user_prototypes, dtype=np.float32)
    shared_prototypes = np.asarray(shared_prototypes, dtype=np.float32)
    interaction_count = np.asarray(interaction_count, dtype=np.float32)

    hot = user_idx.astype(np.int64)
    assert hot.shape == (B,) and np.unique(hot).size == B, \
        "kernel assumes unique user indices"
    cold_mask = np.ones(U, dtype=bool)
    cold_mask[hot] = False
    cold = np.nonzero(cold_mask)[0]
    cold_splits = np.array_split(cold, N_CORES)

    shared2d = np.ascontiguousarray(shared_prototypes.reshape(P, D))

    in_maps = []
    for k in range(N_CORES):
        sl = slice(k * BC, (k + 1) * BC)
        hot_k = hot[sl]
        ck = cold_splits[k]
        feat_t = np.ascontiguousarray(
            features[sl].transpose(0, 2, 1)).reshape(T, 128, D, S)
        sm_k = np.ascontiguousarray(
            success_mask[sl].reshape(T, 128, S).transpose(1, 0, 2))
        cnt_k = np.ascontiguousarray(
            interaction_count[hot_k].reshape(T, 128).T)
        up_k = user_prototypes[hot_k].reshape(T, 128, PD)
        cold_k = user_prototypes[ck].reshape(NCOLD, PD)
        ccnt_k = interaction_count[ck]
        in_maps.append({
            "feat": feat_t, "sm": sm_k, "cnt": cnt_k,
            "up": np.ascontiguousarray(up_k),
            "cold": np.ascontiguousarray(cold_k),
            "ccnt": np.ascontiguousarray(ccnt_k),
            "sh": shared2d,
            "shrep": np.ascontiguousarray(
                np.broadcast_to(shared2d.reshape(1, PD), (128, PD))),
        })

    if _NC is None:
        _NC = _build()
    res = bass_utils.run_bass_kernel_spmd(_NC, in_maps, list(range(N_CORES)))
    LAST_RESULTS = res

    combined = np.empty((B, P, D), dtype=np.float32)
    new_rows = np.empty((B, P, D), dtype=np.float32)
    new_user_prototypes = np.empty((U, P, D), dtype=np.float32)
    new_interaction_count = np.empty(U, dtype=np.float32)
    for k in range(N_CORES):
        sl = slice(k * BC, (k + 1) * BC)
        out = res.results[k]
        combined[sl] = out["comb_o"].reshape(BC, P, D)
        new_rows[sl] = out["newr_o"].reshape(BC, P, D)
        new_user_prototypes[cold_splits[k]] = out["cold_o"].reshape(NCOLD, P, D)
        new_interaction_count[cold_splits[k]] = out["ccnt_o"]
        new_interaction_count[hot[sl]] = out["cnt_o"].T.reshape(BC)
    new_user_prototypes[hot] = new_rows
    new_shared = res.results[0]["sh_o"].reshape(1, P, D).copy()

    return (combined, new_rows, new_user_prototypes, new_shared,
            new_interaction_count)
